# revision 13
# baseline (speedup 1.0000x reference)
"""Trainium2 Bass kernel: single-head GATConv (+ self-loops, segment softmax)
followed by LayerNorm, distributed over 8 NeuronCores.

v2 design (destination-sharded SPMD, bf16 data path):
  * Host: drop self-loops (device adds them via a diagonal matmul from the
    streamed dest-block rows), sort edges by destination, shard destinations
    contiguously across cores.  Within a core, destinations form 128-wide
    blocks; each block's edges split into 8 subgroups (leaky-sign x src
    bank of 25600 rows for int16 dma_gather indices), each padded to x128
    slots with PER-BLOCK widths (max over cores, so one SPMD program
    serves all cores).
  * Softmax factorization: exp(leaky_relu(s+d)) = max(us*ud, vs*vd) with
    us=exp(s), vs=exp(0.2 s), ud=exp(d), vd=exp(0.2 d).  All four factors
    are computed per-node ONCE in phase A and stored in the hext row, so
    phase B needs no transcendentals.  LayerNorm cancels every per-row
    scalar (softmax max-shift and denominator), so no division is needed
    in the non-general (bias=0, gamma=1, beta=0) path.
  * hext row = 128 bf16 cols (256 B, dma_gather's minimum):
    [h(0:64) | 1 | u | v | ud | vd | garbage].  Only cols 0:69 are written.
  * Phase B per chunk of CB blocks: one dma_gather per (subgroup, <=16
    cols); per column one DVE op builds A^T = (iota == dst_rel) * u_col in
    bf16 and one bf16 matmul accumulates into the pos/neg halves of the
    block's PSUM acc; the self-loop lands in the pos half via a diagonal
    A^T scaled max(u*ud, v*vd)/ud; epilogue applies ud/vd column scales
    and LayerNorm.
"""

import math

import numpy as np
import ml_dtypes

import concourse.bacc as bacc
import concourse.bass as bass
import concourse.tile as tile
from concourse import mybir
from concourse.bass_utils import run_bass_kernel_spmd

P = 128
HEXT_W = 128          # bf16 row = 256 B (dma_gather needs 256B-multiple rows)
HEXT_USED = 69        # h(64) + ones + u + v + ud + vd
COL_ONES = 64
COL_U = 65
COL_V = 66
COL_UD = 67
COL_VD = 68
N_BANKS = 4           # dma_gather int16 indices: banks of N_pad//4 <= 32767
N_GROUPS = 2          # positive / negative leaky branch
CALL_COLS = 8         # max 128-slot columns per dma_gather call (1024 descs)

f32 = mybir.dt.float32
bf16 = mybir.dt.bfloat16
i16 = mybir.dt.int16

LEAK = 0.2
LN_EPS = 1e-5

bfdt = ml_dtypes.bfloat16


def _cdiv(a, b):
    return -(-a // b)


def _bf(x):
    return np.asarray(x, dtype=np.float32).astype(bfdt)


# ---------------------------------------------------------------------------
# Host-side preprocessing
# ---------------------------------------------------------------------------

def prep_xT(x, sup):
    """Pad x, permute within super-tiles so contiguous lhsT slices produce
    per-partition-contiguous hext stores, return bf16 [D, N_pad]."""
    N, D = x.shape
    n_sup = _cdiv(N, sup)
    N_pad = n_sup * sup
    xpad = np.zeros((N_pad, D), dtype=np.float32)
    xpad[:N] = x
    MP = sup // P
    xr = xpad.reshape(n_sup, P, MP, D)
    xperm = xr.transpose(0, 2, 1, 3).reshape(N_pad, D)
    return np.ascontiguousarray(_bf(xperm.T)), N_pad


def make_w_aug(W, att_src, att_dst):
    """hi part [D, 68] = [W | ws | .2ws | wd | .2wd], lo part [D, 4] holds
    the bf16 residual of the 4 aux columns (exp args need ~1e-4 accuracy)."""
    D = W.shape[0]
    ws = (W.astype(np.float64) @ att_src.astype(np.float64))
    wd = (W.astype(np.float64) @ att_dst.astype(np.float64))
    aux = np.stack([ws, LEAK * ws, wd, LEAK * wd], axis=1)  # [D, 4]
    aux_hi = _bf(aux)
    aux_lo = _bf(aux - aux_hi.astype(np.float64))
    w_hi = np.concatenate([_bf(W), aux_hi], axis=1)
    return np.ascontiguousarray(w_hi), np.ascontiguousarray(aux_lo)


def prep_edges(x, edge_index, W, att_src, att_dst, n_cores, sup, cb):
    """Shard + sort edges (NO self-loops), split into (sign x bank)
    subgroups per 128-dest block with per-block widths, build device
    slabs.

    Returns (per_core, S, NB, nd_core, layout):
      per_core[c] = dict(idx=int16 [P, W], dr=bf16 [P, TC])
      S: [NB][8] per-(block, group*4+bank) column counts
      layout: dict with n_chunks, cs (per chunk), idx_words (per chunk),
              calls (per chunk: list of (gk, col0_in_chunk, ncols, iw0))
    """
    N, D = x.shape
    assert N % n_cores == 0
    nd_core = N // n_cores
    NB = _cdiv(nd_core, P)
    assert NB % cb == 0
    n_chunks = NB // cb
    n_sup = _cdiv(N, sup)
    N_pad = n_sup * sup
    bank_size = N_pad // N_BANKS
    assert bank_size <= 32768

    ws = (W @ att_src).astype(np.float64)
    wd = (W @ att_dst).astype(np.float64)
    a_s = (x.astype(np.float64) @ ws)
    a_d = (x.astype(np.float64) @ wd)

    src = np.asarray(edge_index[0]).astype(np.int64)
    dst = np.asarray(edge_index[1]).astype(np.int64)

    order = np.argsort(dst, kind="stable")
    s_dst = dst[order]
    s_src = src[order]
    sign = (a_s[s_src] + a_d[s_dst]) <= 0.0   # False=pos group0, True=neg

    bounds = np.searchsorted(s_dst, np.arange(0, N + nd_core, nd_core))

    cnts = np.zeros((n_cores, NB, N_GROUPS * N_BANKS), dtype=np.int64)
    per_core_raw = []
    for c in range(n_cores):
        lo, hi = int(bounds[c]), int(bounds[c + 1])
        d_loc = s_dst[lo:hi] - c * nd_core
        blk = d_loc >> 7
        g = sign[lo:hi].astype(np.int64)
        src_row = (s_src[lo:hi] - c * nd_core) % N   # rotated hext row
        k = src_row // bank_size
        gk = g * N_BANKS + k
        key = blk * 8 + gk
        cnts[c] += np.bincount(key, minlength=NB * 8).reshape(NB, 8)
        per_core_raw.append((d_loc, src_row % bank_size, key))

    S = _cdiv(cnts.max(axis=0), P)               # [NB, 8] per-block widths

    # device column order: chunk-major, then gk, then block, then col
    col_start = np.zeros((NB, 8), dtype=np.int64)
    cs = []
    idx_words = []
    calls_per_chunk = []
    tot = 0
    iw = 0
    for ch in range(n_chunks):
        ch_cols = 0
        calls = []
        for gk in range(8):
            gk_col0 = ch_cols
            for b in range(ch * cb, (ch + 1) * cb):
                col_start[b, gk] = tot + ch_cols
                ch_cols += int(S[b, gk])
            ncols_gk = ch_cols - gk_col0
            c0 = gk_col0
            while ncols_gk > 0:
                cc = min(CALL_COLS, ncols_gk)
                calls.append((gk, c0, cc, iw))
                iw += cc * 8           # int16 words per partition per call
                c0 += cc
                ncols_gk -= cc
        cs.append(ch_cols)
        idx_words.append(iw)
        calls_per_chunk.append(calls)
        tot += ch_cols
    TC = tot
    W_words = iw

    per_core = []
    for c in range(n_cores):
        d_loc, src_bank_row, key = per_core_raw[c]
        order2 = np.argsort(key, kind="stable")
        d2 = d_loc[order2]
        s2 = src_bank_row[order2]
        key2 = key[order2]
        starts = np.zeros(NB * 8 + 1, dtype=np.int64)
        starts[1:] = np.cumsum(np.bincount(key2, minlength=NB * 8))
        pos_in = np.arange(len(key2)) - starts[key2]
        b2 = key2 // 8
        gk2 = key2 % 8
        col = col_start[b2, gk2] + (pos_in >> 7)
        lane = pos_in & 127
        dr = np.full((P, TC), -1.0, dtype=np.float32)
        slot = np.zeros((P, TC), dtype=np.int64)
        dr[lane, col] = (d2 & 127).astype(np.float32)
        slot[lane, col] = s2
        # pack idx: per call, flat order i = col*128 + lane, wrapped into 16
        # partitions (i%16, i//16), replicated across the 8 gpsimd cores
        idx = np.zeros((16, W_words), dtype=np.int16)
        for ch in range(n_chunks):
            base = 0 if ch == 0 else int(np.sum(cs[:ch]))
            for (gk, c0, cc, iw0) in calls_per_chunk[ch]:
                flat = slot[:, base + c0:base + c0 + cc].T.reshape(-1)
                n = flat.shape[0]
                ar = np.arange(n)
                idx[ar % 16, iw0 + ar // 16] = (
                    flat.astype(np.uint16).view(np.int16))
        per_core.append({
            "idx": np.ascontiguousarray(np.tile(idx, (8, 1))),
            "dr": np.ascontiguousarray(dr.astype(np.float32)),
        })

    layout = dict(n_chunks=n_chunks, cs=[int(v) for v in cs],
                  idx_words=[int(v) for v in idx_words],
                  calls=calls_per_chunk, W_words=W_words, TC=TC,
                  bank_size=bank_size)
    return per_core, S, NB, nd_core, layout


# ---------------------------------------------------------------------------
# Device program
# ---------------------------------------------------------------------------

def build_program(N_pad, D, NB, S, CB, SUP, layout, w_hi, w_lo, general,
                  ln_bias=None, ln_gamma=None, ln_beta=None, n_queues=4):
    assert NB % CB == 0
    assert SUP % P == 0 and N_pad % SUP == 0
    bank_size = layout["bank_size"]
    MP = SUP // P
    n_sup = N_pad // SUP
    n_chunks = layout["n_chunks"]
    cs = layout["cs"]
    CS_MAX = max(cs)
    W_words = layout["W_words"]
    NCOL = 65 if general else 64      # matmul rhs width (65 adds ones col)
    AUXW = w_hi.shape[1]              # 68
    LOW = w_lo.shape[1]               # 4

    nc = bacc.Bacc(num_swdge_queues=n_queues)
    xT_d = nc.declare_dram_parameter("xT", [D, N_pad], bf16, isOutput=False)
    idx_d = nc.declare_dram_parameter("idx", [P, W_words], i16, isOutput=False)
    dr_d = nc.declare_dram_parameter("dr", [P, layout["TC"]], f32,
                                     isOutput=False)
    out_d = nc.declare_dram_parameter("out", [NB * P, D], f32, isOutput=True)
    hext = nc.dram_tensor("hext", [N_pad, HEXT_W], bf16)

    w_hi_t = nc.inline_tensor(w_hi, "w_hi")
    w_lo_t = nc.inline_tensor(w_lo, "w_lo")
    iota_np = np.broadcast_to(
        np.arange(P, dtype=np.float32), (P, P)).copy()
    iota_t = nc.inline_tensor(_bf(iota_np), "iota_rows")
    diag_np = np.arange(P, dtype=np.float32).reshape(P, 1)
    diag_t = nc.inline_tensor(diag_np, "diag_rows")
    if general:
        def _rep(v):
            return np.ascontiguousarray(np.broadcast_to(
                np.asarray(v, dtype=np.float32).reshape(1, D), (P, D)))
        bias_t = nc.inline_tensor(_rep(ln_bias), "ln_bias")
        gamma_t = nc.inline_tensor(_rep(ln_gamma), "ln_gamma")
        beta_t = nc.inline_tensor(_rep(ln_beta), "ln_beta")

    with tile.TileContext(nc) as tc:
        with tc.tile_pool(name="const", bufs=1) as cpool:
            iota_sb = cpool.tile([P, P], bf16, tag="c_iota")
            nc.sync.dma_start(out=iota_sb[:], in_=iota_t[:])
            diag_sb = cpool.tile([P, 1], f32, tag="c_diag")
            nc.sync.dma_start(out=diag_sb[:], in_=diag_t[:])
            whi_sb = cpool.tile([D, AUXW], bf16, tag="c_whi")
            nc.sync.dma_start(out=whi_sb[:], in_=w_hi_t[:])
            wlo_sb = cpool.tile([D, LOW], bf16, tag="c_wlo")
            nc.sync.dma_start(out=wlo_sb[:], in_=w_lo_t[:])
            eps_sb = cpool.tile([P, 1], f32, tag="c_eps")
            nc.vector.memset(eps_sb[:], LN_EPS)
            if general:
                bias_sb = cpool.tile([P, D], f32, tag="c_bias")
                nc.sync.dma_start(out=bias_sb[:], in_=bias_t[:])
                gamma_sb = cpool.tile([P, D], f32, tag="c_gamma")
                nc.sync.dma_start(out=gamma_sb[:], in_=gamma_t[:])
                beta_sb = cpool.tile([P, D], f32, tag="c_beta")
                nc.sync.dma_start(out=beta_sb[:], in_=beta_t[:])

            # ---------------- Phase A ------------------------------------
            with tc.tile_pool(name="pa_x", bufs=3) as pa_x, \
                 tc.tile_pool(name="pa_ps", bufs=4, space="PSUM") as pa_ps, \
                 tc.tile_pool(name="pa_h", bufs=3) as pa_h:
                for gsup in range(n_sup):
                    xt_sb = pa_x.tile([D, SUP], bf16)
                    nc.sync.dma_start(
                        out=xt_sb[:], in_=xT_d[:, gsup * SUP:(gsup + 1) * SUP])
                    hx_sb = pa_h.tile([P, MP, HEXT_USED], bf16)
                    for q in range(MP // 4):
                        ps = pa_ps.tile([P, 4, AUXW + LOW], f32)
                        for kk in range(4):
                            m = q * 4 + kk
                            nc.tensor.matmul(
                                ps[:, kk, 0:AUXW],
                                lhsT=xt_sb[:, m * P:(m + 1) * P],
                                rhs=whi_sb[:],
                                start=True, stop=True,
                            )
                            nc.tensor.matmul(
                                ps[:, kk, AUXW:AUXW + LOW],
                                lhsT=xt_sb[:, m * P:(m + 1) * P],
                                rhs=wlo_sb[:],
                                start=True, stop=True,
                            )
                        # aux = hi + lo residual (exp args); PSUM ops may
                        # read only one PSUM operand, so stage lo in SBUF
                        lo_sb = pa_h.tile([P, 4, LOW], f32)
                        nc.scalar.copy(
                            out=lo_sb[:], in_=ps[:, :, AUXW:AUXW + LOW])
                        nc.vector.tensor_add(
                            out=ps[:, :, 64:68],
                            in0=ps[:, :, 64:68], in1=lo_sb[:])
                        sl = slice(q * 4, (q + 1) * 4)
                        if q % 2 == 0:
                            nc.vector.tensor_copy(
                                out=hx_sb[:, sl, 0:64], in_=ps[:, :, 0:64])
                        else:
                            nc.scalar.copy(
                                out=hx_sb[:, sl, 0:64], in_=ps[:, :, 0:64])
                        nc.scalar.activation(
                            out=hx_sb[:, sl, COL_U:COL_VD + 1],
                            in_=ps[:, :, 64:68],
                            func=mybir.ActivationFunctionType.Exp)
                    nc.vector.memset(hx_sb[:, :, COL_ONES:COL_ONES + 1], 1.0)
                    nc.sync.dma_start(
                        out=hext[gsup * SUP:(gsup + 1) * SUP,
                                 0:HEXT_USED].rearrange(
                            "(p m) c -> p m c", m=MP),
                        in_=hx_sb[:],
                    )

            # ---------------- Phase B ------------------------------------
            with tc.tile_pool(name="pb_io", bufs=2) as pb_io, \
                 tc.tile_pool(name="pb_g", bufs=2) as pb_g, \
                 tc.tile_pool(name="pb_uv", bufs=2) as pb_uv, \
                 tc.tile_pool(name="pb_at", bufs=4) as pb_at, \
                 tc.tile_pool(name="pb_y", bufs=3) as pb_y, \
                 tc.tile_pool(name="pb_sm", bufs=6) as pb_sm, \
                 tc.tile_pool(name="pb_ps", bufs=4, space="PSUM") as pb_ps:
                gcall = 0   # global gather-call counter: keeps queue_num
                # aligned with tile's 8 round-robin DMASW sem lanes
                col_base = 0
                for ch in range(n_chunks):
                    CS = cs[ch]
                    iw_base = 0 if ch == 0 else layout["idx_words"][ch - 1]
                    nw = layout["idx_words"][ch] - iw_base
                    idx_sb = pb_io.tile([P, max(8, nw)], i16)
                    nc.sync.dma_start(
                        out=idx_sb[:, 0:nw],
                        in_=idx_d[:, iw_base:iw_base + nw])
                    dr_sb = pb_io.tile([P, CS_MAX], f32)
                    nc.sync.dma_start(
                        out=dr_sb[:, 0:CS],
                        in_=dr_d[:, col_base:col_base + CS])
                    hb_sb = pb_io.tile([P, CB, HEXT_USED], bf16)
                    nc.sync.dma_start(
                        out=hb_sb[:],
                        in_=hext[ch * CB * P:(ch + 1) * CB * P,
                                 0:HEXT_USED].rearrange(
                            "(b p) c -> p b c", p=P))

                    G = pb_g.tile([P, CS_MAX, HEXT_W], bf16)
                    hbx = pb_io.tile([P, CB, 4], f32)
                    nc.vector.tensor_copy(
                        out=hbx[:], in_=hb_sb[:, :, COL_U:COL_VD + 1])
                    # per-dest self-loop scale max(u*ud, v*vd)/ud for all
                    # CB blocks at once (lands in the ud-scaled pos half)
                    wsf = pb_io.tile([P, CB], f32)
                    nc.vector.reciprocal(wsf[:], hbx[:, :, 2])
                    nc.vector.tensor_mul(
                        out=wsf[:], in0=wsf[:], in1=hbx[:, :, 1])
                    nc.vector.tensor_mul(
                        out=wsf[:], in0=wsf[:], in1=hbx[:, :, 3])
                    nc.vector.tensor_max(
                        out=wsf[:], in0=wsf[:], in1=hbx[:, :, 0])
                    for (gk, c0, ncols, iw0) in layout["calls"][ch]:
                        kbank = gk % N_BANKS
                        nidx = ncols * P
                        nc.gpsimd.dma_gather(
                            out_ap=G[:, c0:c0 + ncols, :],
                            in_ap=hext[kbank * bank_size:
                                       (kbank + 1) * bank_size, :],
                            idxs_ap=idx_sb[:, iw0 - iw_base:
                                           iw0 - iw_base + nidx // 16],
                            num_idxs=nidx, num_idxs_reg=nidx,
                            elem_size=HEXT_W,
                            queue_num=gcall % n_queues)
                        gcall += 1

                    # f32 copies of the per-slot u/v factors (is_equal
                    # tensor_scalar requires f32 scalar operands)
                    u_t = pb_uv.tile([P, CS_MAX], f32)
                    nc.vector.tensor_copy(
                        out=u_t[:, 0:CS], in_=G[:, 0:CS, COL_U])
                    v_t = pb_uv.tile([P, CS_MAX], f32)
                    nc.vector.tensor_copy(
                        out=v_t[:, 0:CS], in_=G[:, 0:CS, COL_V])

                    for bb in range(CB):
                        b = ch * CB + bb
                        acc = pb_ps.tile([P, 2 * NCOL], f32)
                        # column ranges of this block within the chunk
                        for g in range(N_GROUPS):
                            scal_t = u_t if g == 0 else v_t
                            half = slice(0, NCOL) if g == 0 else \
                                slice(NCOL, 2 * NCOL)
                            colss = []
                            run = 0
                            for gk in range(8):
                                if gk // N_BANKS == g:
                                    off = run
                                    for b2 in range(ch * CB, b):
                                        off += int(S[b2, gk])
                                    colss.extend(
                                        range(off, off + int(S[b, gk])))
                                run += sum(int(S[b2, gk]) for b2 in
                                           range(ch * CB, (ch + 1) * CB))
                            n_mm = len(colss) + (1 if g == 0 else 0)
                            ii = 0
                            for cc in colss:
                                at = pb_at.tile([P, P], bf16)
                                nc.vector.tensor_scalar(
                                    out=at[:], in0=iota_sb[:],
                                    scalar1=dr_sb[:, cc:cc + 1],
                                    scalar2=scal_t[:, cc:cc + 1],
                                    op0=mybir.AluOpType.is_equal,
                                    op1=mybir.AluOpType.mult,
                                )
                                nc.tensor.matmul(
                                    acc[:, half], lhsT=at[:],
                                    rhs=G[:, cc, 0:NCOL],
                                    start=(ii == 0),
                                    stop=(ii == n_mm - 1),
                                )
                                ii += 1
                            if g == 0:
                                # self-loop: diag(max(u*ud, v*vd)/ud) @ h_blk
                                at = pb_at.tile([P, P], bf16)
                                nc.vector.tensor_scalar(
                                    out=at[:], in0=iota_sb[:],
                                    scalar1=diag_sb[:],
                                    scalar2=wsf[:, bb:bb + 1],
                                    op0=mybir.AluOpType.is_equal,
                                    op1=mybir.AluOpType.mult,
                                )
                                nc.tensor.matmul(
                                    acc[:, half], lhsT=at[:],
                                    rhs=hb_sb[:, bb, 0:NCOL],
                                    start=(ii == 0), stop=True)

                        # epilogue: y0 = ud*accp + vd*accn (+ den path)
                        t2 = pb_y.tile([P, D], f32)
                        nc.scalar.activation(
                            out=t2[:], in_=acc[:, NCOL:NCOL + D],
                            func=mybir.ActivationFunctionType.Copy,
                            scale=hbx[:, bb, 3:4])
                        y0 = pb_y.tile([P, D], f32)
                        nc.vector.scalar_tensor_tensor(
                            out=y0[:], in0=acc[:, 0:D],
                            scalar=hbx[:, bb, 2:3],
                            in1=t2[:],
                            op0=mybir.AluOpType.mult,
                            op1=mybir.AluOpType.add,
                        )
                        if general:
                            den = pb_sm.tile([P, 1], f32)
                            nc.scalar.activation(
                                out=den[:], in_=acc[:, 64:65],
                                func=mybir.ActivationFunctionType.Copy,
                                scale=hbx[:, bb, 2:3])
                            den2 = pb_sm.tile([P, 1], f32)
                            nc.scalar.activation(
                                out=den2[:], in_=acc[:, NCOL + 64:NCOL + 65],
                                func=mybir.ActivationFunctionType.Copy,
                                scale=hbx[:, bb, 3:4])
                            nc.vector.tensor_add(
                                out=den[:], in0=den[:], in1=den2[:])
                            rec = pb_sm.tile([P, 1], f32)
                            nc.vector.reciprocal(rec[:], den[:])
                            nc.vector.tensor_scalar_mul(
                                out=y0[:], in0=y0[:], scalar1=rec[:])
                            nc.vector.tensor_add(
                                out=y0[:], in0=y0[:], in1=bias_sb[:])
                        st = pb_sm.tile([P, 6], f32)
                        nc.vector.bn_stats(out=st[:], in_=y0[:])
                        mv = pb_sm.tile([P, 2], f32)
                        nc.vector.bn_aggr(out=mv[:], in_=st[:])
                        sd = pb_sm.tile([P, 1], f32)
                        nc.scalar.activation(
                            out=sd[:], in_=mv[:, 1:2],
                            func=mybir.ActivationFunctionType.Sqrt,
                            bias=eps_sb[:])
                        nc.vector.reciprocal(sd[:], sd[:])
                        if bb == 0:
                            ytile = pb_y.tile([P, CB, D], f32)
                        nc.vector.tensor_scalar(
                            out=ytile[:, bb, :], in0=y0[:],
                            scalar1=mv[:, 0:1], scalar2=sd[:],
                            op0=mybir.AluOpType.subtract,
                            op1=mybir.AluOpType.mult,
                        )
                        if general:
                            nc.vector.tensor_mul(
                                out=ytile[:, bb, :], in0=ytile[:, bb, :],
                                in1=gamma_sb[:])
                            nc.vector.tensor_add(
                                out=ytile[:, bb, :], in0=ytile[:, bb, :],
                                in1=beta_sb[:])
                    nc.sync.dma_start(
                        out=out_d[ch * CB * P:(ch + 1) * CB * P, :].rearrange(
                            "(b p) c -> p b c", p=P),
                        in_=ytile[:])
                    col_base += CS
    nc.finalize()
    return nc


# ---------------------------------------------------------------------------
# Entry point
# ---------------------------------------------------------------------------

N_CORES = 8
SUP_DEFAULT = 2048

LAST_RESULTS = None


def _pick_cb(NB):
    for cb in (7, 8, 6, 5, 4, 2):
        if NB % cb == 0:
            return cb
    return 1


def build_all(x, edge_index, W, att_src, att_dst, bias, gamma, beta):
    x = np.asarray(x, dtype=np.float32)
    W = np.asarray(W, dtype=np.float32)
    att_src = np.asarray(att_src, dtype=np.float32)
    att_dst = np.asarray(att_dst, dtype=np.float32)
    bias = np.asarray(bias, dtype=np.float32)
    gamma = np.asarray(gamma, dtype=np.float32)
    beta = np.asarray(beta, dtype=np.float32)
    N, D = x.shape
    nd_core = N // N_CORES
    NB = _cdiv(nd_core, P)
    CB = _pick_cb(NB)

    xTs = []
    for c in range(N_CORES):
        xT, N_pad = prep_xT(np.roll(x, -c * nd_core, axis=0), SUP_DEFAULT)
        xTs.append(xT)
    per_core, S, NB, nd_core, layout = prep_edges(
        x, edge_index, W, att_src, att_dst, N_CORES, SUP_DEFAULT, CB)
    w_hi, w_lo = make_w_aug(W, att_src, att_dst)
    general = not (
        np.all(bias == 0.0) and np.all(gamma == 1.0) and np.all(beta == 0.0))

    nc = build_program(
        N_pad, D, NB, S, CB, SUP_DEFAULT, layout, w_hi, w_lo, general,
        ln_bias=bias, ln_gamma=gamma, ln_beta=beta)

    in_maps = []
    for c in range(N_CORES):
        in_maps.append({"xT": xTs[c], "idx": per_core[c]["idx"],
                        "dr": per_core[c]["dr"]})
    return nc, in_maps, nd_core, S


def kernel(x, edge_index, W, att_src, att_dst, bias, gamma, beta):
    global LAST_RESULTS
    nc, in_maps, nd_core, S = build_all(
        x, edge_index, W, att_src, att_dst, bias, gamma, beta)
    res = run_bass_kernel_spmd(nc, in_maps, list(range(N_CORES)))
    LAST_RESULTS = res
    out = np.concatenate(
        [res.results[c]["out"][:nd_core] for c in range(N_CORES)], axis=0)
    return out.astype(np.float32)
